# revision 11
# baseline (speedup 1.0000x reference)
"""Trainium2 Bass kernel for nn_GroupedQueryAttention_86380382257377.

Math note: the reference einsums collapse dramatically.
  scores = einsum('bqghd,bkgd->bqhg', q, k)  reduces over BOTH key pos and d,
  so only ksum[b,g,:] = sum_s k[b,s,g,:] is needed:
      scores[b,q,h,g] = x[b,q,:] . (Wq_blk[g,h] @ ksum[b,g]) / sqrt(D)
  out = einsum('bqhg,bsgd->bsgd', w, v) = wsum[b,g] * v[b,s,g,:]
  with wsum[b,g] = sum_{q,h} softmax_g(scores)[b,q,h,g], so
      out[b] = x[b] @ M[b] + cvec[b],
      M[b]   = sum_g wsum[b,g] * (Wv_g @ Wo_g),
      cvec[b]= sum_g wsum[b,g] * (bv_g @ Wo_g) + bo.

Sharding over 8 cores: core c owns group c for the Wq/Wk shards (one small
AllGather of the [D,B,H] wq_eff vectors) and owns output columns
[c*64,(c+1)*64) for the Wv@Wo / x@M stage (xT and WvT replicated).
"""

import numpy as np

B, S, D, G, H = 2, 2048, 512, 8, 4
N_CORES = 8
FSL = D // N_CORES  # 64 output columns per core
P = 128
DC = D // P  # 4
JC = S // P  # 16  (128-row score chunks over the full sequence)
SC = S // 512  # 4  (512-col moving chunks for the out matmul)
INV_SQRT_D = 1.0 / float(np.sqrt(D))

_cache = {}


def _build_nc():
    import concourse.bass as bass
    import concourse.mybir as mybir
    import concourse.tile as tile
    from concourse import bacc

    dt = mybir.dt.float32
    nc = bacc.Bacc(None, num_devices=N_CORES)

    # ---- kernel I/O (per-core views, host-prepared) ----
    xT_d = nc.dram_tensor("xT", [D, B, S], dt, kind="ExternalInput")      # [a, b, s]
    wvT_d = nc.dram_tensor("wvT", [G, D, D], dt, kind="ExternalInput")    # [g, e, a]
    wo_d = nc.dram_tensor("wo_sl", [G, D, FSL], dt, kind="ExternalInput")  # [g, e, f]
    wq_d = nc.dram_tensor("wqT", [D, H, D], dt, kind="ExternalInput")     # [e, h, a]
    wk_d = nc.dram_tensor("wk", [D, D], dt, kind="ExternalInput")         # [d, e]
    bk_d = nc.dram_tensor("bk_c", [D], dt, kind="ExternalInput")
    bq_d = nc.dram_tensor("bq_c", [H * D], dt, kind="ExternalInput")      # [h*512+e]
    bv_d = nc.dram_tensor("bv", [G * D], dt, kind="ExternalInput")
    bo_d = nc.dram_tensor("bo_sl", [FSL], dt, kind="ExternalInput")
    out_d = nc.dram_tensor("outT", [B, FSL, S], dt, kind="ExternalOutput")

    with tile.TileContext(nc) as tc:
        with (
            tc.tile_pool(name="sing", bufs=1) as sing,
            tc.tile_pool(name="wvp", bufs=2) as wvp,
            tc.tile_pool(name="pp", bufs=3, space="PSUM") as pp,
            tc.tile_pool(name="ppP", bufs=4, space="PSUM") as ppP,
            tc.tile_pool(name="dram", bufs=1, space="DRAM") as dram,
        ):
            # ---- persistent SBUF tiles ----
            x_sb = sing.tile([P, DC, B, S], dt)          # 8MB  [a_p, ac, b, s]
            wq_sb = sing.tile([P, DC, H, D], dt)         # 4MB  [e_p, ec, h, a]
            wo_sb = sing.tile([P, G, DC, FSL], dt)       # 1MB  [e_p, g, ec, f]
            wk_sb = sing.tile([P, DC, D], dt)            # 1MB  [d_p, dc, e]
            p_sb = sing.tile([P, DC, G, FSL], dt)        # 1MB  [a_p, ac, g, f]
            m_sb = sing.tile([P, DC, B, FSL], dt)        # .5MB [a_p, ac, b, f]
            out_sb = sing.tile([FSL, B, S], dt)          # 1MB  [f, b, s]
            wqe_all = sing.tile([P, DC, B, G, H], dt)    # .5MB [a_p, ac, b, g, h]
            s1_sb = sing.tile([P, B, JC, G, H], dt)      # .5MB scratch
            s2_sb = sing.tile([P, B, JC, G, H], dt)      # .5MB weights
            tmax = sing.tile([P, B, JC, H], dt)
            tden = sing.tile([P, B, JC, H], dt)
            trec = sing.tile([P, B, JC, H], dt)
            xs_sb = sing.tile([P, DC, B], dt)
            ksum_sb = sing.tile([P, DC, B], dt)          # [e_p, ec, b]
            bk_sb = sing.tile([P, DC], dt)
            bq_sb = sing.tile([P, DC, H], dt)            # [e_p, ec, h]
            bv_sb = sing.tile([P, G * DC], dt)           # [ge_p, ec32]
            bvs_sb = sing.tile([P, B, G * DC], dt)
            bo_sb = sing.tile([FSL, 1], dt)
            ones_sb = sing.tile([P, 1], dt)
            wsum_sb = sing.tile([1, B, G], dt)
            wsum_bc = sing.tile([P, B, G], dt)
            bqd_bc = sing.tile([P, B, G, H], dt)
            cvec_sb = sing.tile([FSL, B], dt)

            # ---- internal DRAM (collective bounce + broadcast) ----
            CHUNK = D * B * H + H * B  # 4096 wq_eff + 8 bq_dot
            wq_bounce = dram.tile([CHUNK], dt)
            wq_gath = dram.tile([G * CHUNK], dt)
            wsum_dd = dram.tile([B, G], dt)

            # ---- input DMAs (priority order = program order) ----
            for dc in range(DC):
                nc.sync.dma_start(
                    out=x_sb[:, dc, :, :], in_=xT_d[dc * P:(dc + 1) * P, :, :]
                )
            nc.sync.dma_start(
                out=wk_sb[:, :, :],
                in_=wk_d.rearrange("(dc p) e -> p dc e", p=P),
            )
            nc.sync.dma_start(
                out=bk_sb[:, :], in_=bk_d.rearrange("(ec p) -> p ec", p=P)
            )
            for h in range(H):
                nc.sync.dma_start(
                    out=bq_sb[:, :, h],
                    in_=bq_d[h * D:(h + 1) * D].rearrange("(ec p) -> p ec", p=P),
                )
            nc.sync.dma_start(
                out=wq_sb[:, :, :, :],
                in_=wq_d.rearrange("(ec p) h a -> p ec h a", p=P),
            )
            for g in range(G):
                nc.sync.dma_start(
                    out=wo_sb[:, g, :, :],
                    in_=wo_d[g, :, :].rearrange("(ec p) f -> p ec f", p=P),
                )
            nc.sync.dma_start(
                out=bv_sb[:, :], in_=bv_d.rearrange("(ec p) -> p ec", p=P)
            )
            nc.sync.dma_start(
                out=bo_sb[:, :], in_=bo_d.rearrange("(f o) -> f o", o=1)
            )
            nc.vector.memset(ones_sb[:, :], 1.0)

            # ---- A. xs[b,d] = sum_s x  (reduce innermost S) ----
            for dc in range(DC):
                nc.vector.tensor_reduce(
                    out=xs_sb[:, dc, :],
                    in_=x_sb[:, dc, :, :],
                    axis=mybir.AxisListType.X,
                    op=mybir.AluOpType.add,
                )

            # ---- B. ksumT[e,b] = Wk_c^T xs + S*bk  ----
            nc.vector.tensor_scalar_mul(bk_sb[:, :], bk_sb[:, :], float(S))
            psum_k = pp.tile([P, DC, B], dt, tag="big")
            for ec in range(DC):
                for dc in range(DC):
                    nc.tensor.matmul(
                        psum_k[:, ec, :],
                        lhsT=wk_sb[:, dc, ec * P:(ec + 1) * P],
                        rhs=xs_sb[:, dc, :],
                        start=(dc == 0),
                        stop=(dc == DC - 1),
                    )
            for ec in range(DC):
                nc.vector.tensor_scalar_add(
                    ksum_sb[:, ec, :], psum_k[:, ec, :], bk_sb[:, ec:ec + 1]
                )

            # ---- C. wq_eff[a,(b)] per (h, ac); bq_dot[h,b] ----
            psum_wq = pp.tile([P, H, DC, B], dt, tag="big")
            for h in range(H):
                for ac in range(DC):
                    for ec in range(DC):
                        nc.tensor.matmul(
                            psum_wq[:, h, ac, :],
                            lhsT=wq_sb[:, ec, h, ac * P:(ac + 1) * P],
                            rhs=ksum_sb[:, ec, :],
                            start=(ec == 0),
                            stop=(ec == DC - 1),
                        )
            psum_bqd = pp.tile([B, H], dt, tag="big")
            for ec in range(DC):
                nc.tensor.matmul(
                    psum_bqd[:, :],
                    lhsT=ksum_sb[:, ec, :],
                    rhs=bq_sb[:, ec, :],
                    start=(ec == 0),
                    stop=(ec == DC - 1),
                )
            # stage psum -> sbuf (layout [p, ac, b, h]) -> flat dram bounce
            wqe_loc = sing.tile([P, DC, B, H], dt)
            bqd_loc = sing.tile([B, H], dt)
            nc.vector.tensor_copy(
                wqe_loc[:, :, :, :].rearrange("p ac b h -> p h ac b"),
                psum_wq[:, :, :, :],
            )
            nc.vector.tensor_copy(bqd_loc[:, :], psum_bqd[:, :])
            nc.sync.dma_start(
                out=wq_bounce[0:D * B * H].rearrange(
                    "(p ac b h) -> p ac b h", p=P, ac=DC, b=B
                ),
                in_=wqe_loc[:, :, :, :],
            )
            nc.sync.dma_start(
                out=wq_bounce[D * B * H:CHUNK].rearrange("(b h) -> b h", b=B),
                in_=bqd_loc[:, :],
            )

            # ---- D. AllGather of (wq_eff, bq_dot) ----
            nc.gpsimd.collective_compute(
                "AllGather",
                mybir.AluOpType.bypass,
                replica_groups=[list(range(N_CORES))],
                ins=[wq_bounce[:].opt()],
                outs=[wq_gath[:].opt()],
            )

            # ---- E. spread gathered results ----
            gap = wq_gath[:]
            for b in range(B):
                for ac in range(DC):
                    nc.sync.dma_start(
                        out=wqe_all[:, ac, b, :, :].opt(),
                        in_=bass.AP(
                            tensor=gap.tensor,
                            offset=gap.offset + ac * B * H + b * H,
                            ap=[[DC * B * H, P], [CHUNK, G], [1, H]],
                        ),
                    )
            for b in range(B):
                nc.sync.dma_start(
                    out=bqd_bc[:, b, :, :],
                    in_=bass.AP(
                        tensor=gap.tensor,
                        offset=gap.offset + D * B * H + b * H,
                        ap=[[0, P], [CHUNK, G], [1, H]],
                    ),
                )
            nc.vector.tensor_scalar_mul(
                bqd_bc[:, :, :, :], bqd_bc[:, :, :, :], INV_SQRT_D
            )

            # ---- F. scores + softmax + wsum (full sequence, every core) ----
            for b in range(B):
                psum_s = pp.tile([P, JC, G, H], dt, tag="big")
                for j in range(JC):
                    for dc in range(DC):
                        nc.tensor.matmul(
                            psum_s[:, j, :, :],
                            lhsT=x_sb[:, dc, b, j * P:(j + 1) * P],
                            rhs=wqe_all[:, dc, b, :, :],
                            start=(dc == 0),
                            stop=(dc == DC - 1),
                        )
                # t = scores*inv_sqrt_d + bqd   (into s1)
                bqd_b = bqd_bc[:, b, :, :]
                nc.vector.scalar_tensor_tensor(
                    out=s1_sb[:, b, :, :, :],
                    in0=psum_s[:, :, :, :],
                    scalar=INV_SQRT_D,
                    in1=bass.AP(
                        tensor=bqd_b.tensor,
                        offset=bqd_b.offset,
                        ap=[list(bqd_b.ap[0]), [0, JC]] + list(bqd_b.ap[1:]),
                    ),
                    op0=mybir.AluOpType.mult,
                    op1=mybir.AluOpType.add,
                )
                # row max over g (innermost via stride permute)
                nc.vector.tensor_reduce(
                    out=tmax[:, b, :, :],
                    in_=s1_sb[:, b, :, :, :].rearrange("p j g h -> p j h g"),
                    axis=mybir.AxisListType.X,
                    op=mybir.AluOpType.max,
                )
                tmax_b = tmax[:, b, :, :]
                nc.vector.tensor_tensor(
                    out=s2_sb[:, b, :, :, :].rearrange("p j g h -> p j h g"),
                    in0=s1_sb[:, b, :, :, :].rearrange("p j g h -> p j h g"),
                    in1=bass.AP(
                        tensor=tmax_b.tensor,
                        offset=tmax_b.offset,
                        ap=list(tmax_b.ap) + [[0, G]],
                    ),
                    op=mybir.AluOpType.subtract,
                )
                nc.scalar.activation(
                    out=s1_sb[:, b, :, :, :],
                    in_=s2_sb[:, b, :, :, :],
                    func=mybir.ActivationFunctionType.Exp,
                )
                nc.vector.tensor_reduce(
                    out=tden[:, b, :, :],
                    in_=s1_sb[:, b, :, :, :].rearrange("p j g h -> p j h g"),
                    axis=mybir.AxisListType.X,
                    op=mybir.AluOpType.add,
                )
                nc.vector.reciprocal(trec[:, b, :, :], tden[:, b, :, :])
                trec_b = trec[:, b, :, :]
                nc.vector.tensor_tensor(
                    out=s2_sb[:, b, :, :, :].rearrange("p j g h -> p j h g"),
                    in0=s1_sb[:, b, :, :, :].rearrange("p j g h -> p j h g"),
                    in1=bass.AP(
                        tensor=trec_b.tensor,
                        offset=trec_b.offset,
                        ap=list(trec_b.ap) + [[0, G]],
                    ),
                    op=mybir.AluOpType.mult,
                )
                # wsum partial: ones^T @ weights -> [1, JC*G*H], reduce (j,h)
                psum_ws = pp.tile([1, JC * G * H], dt, tag="big")
                nc.tensor.matmul(
                    psum_ws[:, :],
                    lhsT=ones_sb[:, :],
                    rhs=s2_sb[:, b, :, :, :],
                    start=True,
                    stop=True,
                )
                # view [1, (g), (j), (h)] with g kept, (j,h) reduced
                psv = psum_ws[:, :].rearrange(
                    "p (j g h) -> p g j h", j=JC, g=G, h=H
                )
                nc.vector.tensor_reduce(
                    out=wsum_sb[:, b, :],
                    in_=psv,
                    axis=mybir.AxisListType.XY,
                    op=mybir.AluOpType.add,
                )

            # broadcast wsum to all partitions via DRAM
            nc.sync.dma_start(out=wsum_dd[:, :], in_=wsum_sb[:, :, :])
            wsrc = wsum_dd[:, :]
            nc.sync.dma_start(
                out=wsum_bc[:, :, :],
                in_=bass.AP(
                    tensor=wsrc.tensor,
                    offset=wsrc.offset,
                    ap=[[0, P]] + list(wsrc.ap),
                ),
            )

            # ---- G. P_g = Wv_g @ Wo_g[:, fsl]  (all groups, f-slice) ----
            for g in range(G):
                wv_g = wvp.tile([P, DC, D], dt)
                nc.sync.dma_start(
                    out=wv_g[:, :, :],
                    in_=wvT_d[g, :, :].rearrange("(ec p) a -> p ec a", p=P),
                )
                for ac in range(DC):
                    psum_p = ppP.tile([P, FSL], dt)
                    for ec in range(DC):
                        nc.tensor.matmul(
                            psum_p[:, :],
                            lhsT=wv_g[:, ec, ac * P:(ac + 1) * P],
                            rhs=wo_sb[:, g, ec, :],
                            start=(ec == 0),
                            stop=(ec == DC - 1),
                        )
                    nc.vector.tensor_copy(p_sb[:, ac, g, :], psum_p[:, :])

            # ---- H. M[b] = sum_g wsum[b,g] * P_g ----
            for b in range(B):
                nc.vector.tensor_scalar_mul(
                    m_sb[:, :, b, :], p_sb[:, :, 0, :], wsum_bc[:, b, 0:1]
                )
                for g in range(1, G):
                    nc.vector.scalar_tensor_tensor(
                        out=m_sb[:, :, b, :],
                        in0=p_sb[:, :, g, :],
                        scalar=wsum_bc[:, b, g:g + 1],
                        in1=m_sb[:, :, b, :],
                        op0=mybir.AluOpType.mult,
                        op1=mybir.AluOpType.add,
                    )

            # ---- I. cvec[b] = sum_g wsum[b,g] * (bv_g @ Wo_g[:,fsl]) + bo ----
            for b in range(B):
                wsb = wsum_bc[:, b, :]
                nc.vector.tensor_tensor(
                    out=bvs_sb[:, b, :].rearrange("p (g r) -> p g r", g=G),
                    in0=bv_sb[:, :].rearrange("p (g r) -> p g r", g=G),
                    in1=bass.AP(
                        tensor=wsb.tensor,
                        offset=wsb.offset,
                        ap=list(wsb.ap) + [[0, DC]],
                    ),
                    op=mybir.AluOpType.mult,
                )
                psum_cv = pp.tile([FSL, 1], dt, tag="big")
                for ec32 in range(G * DC):
                    nc.tensor.matmul(
                        psum_cv[:, :],
                        lhsT=wo_sb[:, ec32 // DC, ec32 % DC, :],
                        rhs=bvs_sb[:, b, ec32:ec32 + 1],
                        start=(ec32 == 0),
                        stop=(ec32 == G * DC - 1),
                    )
                nc.vector.tensor_tensor(
                    out=cvec_sb[:, b:b + 1],
                    in0=psum_cv[:, :],
                    in1=bo_sb[:, :],
                    op=mybir.AluOpType.add,
                )

            # ---- J. outT[b] = (x[b] @ M[b])^T + cvec ----
            for b in range(B):
                for sc in range(SC):
                    psum_o = pp.tile([FSL, 512], dt, tag="big")
                    for ac in range(DC):
                        nc.tensor.matmul(
                            psum_o[:, :],
                            lhsT=m_sb[:, ac, b, :],
                            rhs=x_sb[:, ac, b, sc * 512:(sc + 1) * 512],
                            start=(ac == 0),
                            stop=(ac == DC - 1),
                        )
                    nc.vector.tensor_scalar_add(
                        out_sb[:, b, sc * 512:(sc + 1) * 512],
                        psum_o[:, :],
                        cvec_sb[:, b:b + 1],
                    )
                nc.sync.dma_start(out=out_d[b, :, :], in_=out_sb[:, b, :])

    nc.compile()
    return nc


def kernel(x, Wq, bq, Wk, bk, Wv, bv, Wo, bo):
    from concourse.bass_utils import run_bass_kernel_spmd

    if "nc" not in _cache:
        _cache["nc"] = _build_nc()
    nc = _cache["nc"]

    x = np.ascontiguousarray(x, dtype=np.float32)
    xT = np.ascontiguousarray(x.transpose(2, 0, 1))                    # [D,B,S]
    wvT = np.ascontiguousarray(
        Wv.astype(np.float32).reshape(D, G, D).transpose(1, 2, 0)      # [g,e,a]
    )
    wo_r = Wo.astype(np.float32).reshape(G, D, D)
    wq_r = Wq.astype(np.float32).reshape(D, G, H, D)
    bq_r = np.ascontiguousarray(bq, dtype=np.float32)
    in_maps = []
    for c in range(N_CORES):
        fs = slice(c * FSL, (c + 1) * FSL)
        in_maps.append({
            "xT": xT,
            "wvT": wvT,
            "wo_sl": np.ascontiguousarray(wo_r[:, :, fs]),
            "wqT": np.ascontiguousarray(wq_r[:, c].transpose(2, 1, 0)),  # [e,h,a]
            "wk": np.ascontiguousarray(Wk[:, c * D:(c + 1) * D].astype(np.float32)),
            "bk_c": np.ascontiguousarray(bk[c * D:(c + 1) * D].astype(np.float32)),
            "bq_c": np.ascontiguousarray(bq_r[c * H * D:(c + 1) * H * D]),
            "bv": np.ascontiguousarray(bv, dtype=np.float32),
            "bo_sl": np.ascontiguousarray(bo[fs].astype(np.float32)),
        })
    res = run_bass_kernel_spmd(nc, in_maps, core_ids=list(range(N_CORES)))
    _cache["last_results"] = res
    outs = [r["outT"] for r in res.results]          # each [B, FSL, S]
    full = np.concatenate(outs, axis=1)              # [B, D, S]
    return np.ascontiguousarray(full.transpose(0, 2, 1)).astype(np.float32)
